# revision 1
# baseline (speedup 1.0000x reference)
"""BitNet-style attention block (ternary-quantized QKV/proj) on 8 Trainium2 cores.

Strategy: data-parallel over batch (16 batches -> 2 per core, no collectives).

v3: all-bf16 matmuls (fp8/DoubleRow measured 1.8x SLOWER per output row on
this hardware), with the pipeline restructured around a saturated PE:
  - Ternary weights are computed host-side (identical float32 comparisons
    with float64-derived scale/threshold, matching the reference's boundary
    decisions exactly) and shipped pre-laid-out in bf16 ({-1,0,1} exact).
  - Q.T/K.T are generated unscaled (SCALE*s^2 folds into the exp scale
    operand) into a feature-major qksb [128, 12, T]; attention uses the
    head-pair disjoint-row-group trick (head 2i on partitions 0:64,
    2i+1 on 64:128; K=64 contraction).
  - exp runs 1024 queries wide ([128, 1024] PSUM spanning 2 banks) on ACT.
  - softmax normalization: v_aug's ones-column yields l as av row 64; the
    four l-rows of a head-pair pack at partition bases 0/32/64/96 so ONE
    [128,512] DVE reciprocal covers them (reciprocal cost is free-size
    driven; the serial [1,512]-at-a-time version cost 3.3us each), then a
    DRAM round-trip broadcast and one multiply per (head, query-half).
  - Emission interleaves Q/K-gen, V-gen and proj matmul blocks into the
    attention kb-loops as PE filler so the PE stays back-to-back (high
    p-state) while ACT grinds the exps.
"""

import os
import sys

import ml_dtypes
import numpy as np

for _p in ("/opt/trn_rl_repo", "/root/.axon_site/_ro/trn_rl_repo"):
    if os.path.isdir(_p) and _p not in sys.path:
        sys.path.insert(0, _p)

import concourse.bass as bass
import concourse.mybir as mybir
import concourse.tile as tile
from concourse import bacc
from concourse.bass_utils import run_bass_kernel_spmd

B, N, C, H = 16, 1024, 768, 12
HD = C // H                    # 64
SCALE = float(HD ** -0.5)      # 0.125
EPS = 1e-5
NCORES = 8
BPC = B // NCORES              # 2 batches per core
T = BPC * N                    # 2048 tokens per core
P = 128
CB = C // P                    # 6 c-blocks of 128
MQK = 2 * CB                   # 12 d-blocks covering Q and K
HP = H // 2                    # 6 head pairs
KB = N // P                    # 8 key blocks per batch
F32 = mybir.dt.float32
BF16 = mybir.dt.bfloat16
AF = mybir.ActivationFunctionType
ALU = mybir.AluOpType

_CACHED_NC = None
_DEBUG = False


def _split_drain_waits(nc):
    """The walrus build in this container accepts only one sync-wait per
    instruction; move extra waits onto preceding single-wait NoOps on the
    same engine (in-order queues make this semantics-preserving)."""
    for fn in nc.m.functions:
        for bb in fn.blocks:
            insts = bb.instructions
            i = 0
            while i < len(insts):
                inst = insts[i]
                si = getattr(inst, "sync_info", None)
                if (
                    si is not None
                    and si.on_wait is not None
                    and len(si.on_wait) > 1
                    and not type(inst).__name__.startswith("InstDMA")
                ):
                    waits = list(si.on_wait)
                    for j, w in enumerate(waits[:-1]):
                        nop = mybir.InstNoOp(
                            name=f"{inst.name}-prewait-{j}", ins=[], outs=[]
                        )
                        nop.engine = inst.engine
                        nop.sync_info = mybir.SyncInfo(on_wait=[w], on_update=[])
                        insts.insert(i, nop)
                        i += 1
                    inst.sync_info = mybir.SyncInfo(
                        on_wait=[waits[-1]], on_update=list(si.on_update)
                    )
                i += 1


def _build_nc(split=True):
    nc = bacc.Bacc(None)

    xT = nc.dram_tensor("xT", [P, CB, T], BF16, kind="ExternalInput")
    wq16 = nc.dram_tensor("wq16", [P, CB, 2 * C], BF16, kind="ExternalInput")
    wv16 = nc.dram_tensor("wv16", [P, CB, C], BF16, kind="ExternalInput")
    wp16 = nc.dram_tensor("wp16", [P, CB, C], BF16, kind="ExternalInput")
    bp = nc.dram_tensor("bp", [C], F32, kind="ExternalInput")
    sq = nc.dram_tensor("sq", [1, 2], F32, kind="ExternalInput")  # [s, SCALE*s^2]
    sp = nc.dram_tensor("sp", [1, 1], F32, kind="ExternalInput")  # [s]
    cz = nc.dram_tensor("cz", [2, N], BF16, kind="ExternalInput")  # row0=0, row1=1
    yT = nc.dram_tensor("yT", [CB, P, T], F32, kind="ExternalOutput")
    if _DEBUG:
        qk_dbg = nc.dram_tensor("qk_dbg", [P, MQK, T], BF16, kind="ExternalOutput")
        va_dbg = nc.dram_tensor("va_dbg", [P, 2 * KB, H, HD + 1], BF16, kind="ExternalOutput")
        out_dbg = nc.dram_tensor("out_dbg", [P, CB, T], BF16, kind="ExternalOutput")
        e_dbg = nc.dram_tensor("e_dbg", [P, 1024], BF16, kind="ExternalOutput")
        av_dbg = nc.dram_tensor("av_dbg", [HD + 1, 512], F32, kind="ExternalOutput")
        linv_dbg = nc.dram_tensor("linv_dbg", [1, 512], F32, kind="ExternalOutput")
        bc_dbg = nc.dram_tensor("bc_dbg", [HD, 512], F32, kind="ExternalOutput")

    with tile.TileContext(nc) as tc:
        with (
            tc.tile_pool(name="constp", bufs=1) as constp,
            tc.tile_pool(name="xp", bufs=1) as xp,
            tc.tile_pool(name="wqp", bufs=1) as wqp,
            tc.tile_pool(name="wvp", bufs=1) as wvp,
            tc.tile_pool(name="wpp", bufs=1) as wpp,
            tc.tile_pool(name="vaugp", bufs=1) as vaugp,
            tc.tile_pool(name="qksp", bufs=1) as qksp,
            tc.tile_pool(name="outp", bufs=1) as outp,
            tc.tile_pool(name="ep", bufs=4) as ep,
            tc.tile_pool(name="bcp", bufs=2) as bcp,
            tc.tile_pool(name="avsp", bufs=4) as avsp,
            tc.tile_pool(name="ystp", bufs=2) as ystp,
            tc.tile_pool(name="psp", bufs=2, space="PSUM") as psp,   # [P,1024] 2-bank
            tc.tile_pool(name="avp", bufs=4, space="PSUM") as avp,   # [65,512] 1-bank
            tc.tile_pool(name="dramls", bufs=4, space="DRAM") as dramls,
        ):
            # ---- scalars / bias / ones ----
            sqb = constp.tile([P, 2], F32, tag="sqb")
            spb = constp.tile([P, 1], F32, tag="spb")
            nc.sync.dma_start(sqb[:], sq[:, :].to_broadcast([P, 2]))
            nc.sync.dma_start(spb[:], sp[:, :].to_broadcast([P, 1]))
            b_sb = constp.tile([P, CB], F32, tag="b_sb")
            nc.sync.dma_start(b_sb[:], bp[:].rearrange("(cb p) -> p cb", p=P))
            ones_col = constp.tile([P, 1], BF16, tag="ones_col")
            nc.sync.dma_start(ones_col[:], cz[1:2, 0:1].to_broadcast([P, 1]))

            # ---- inputs: x on the ACT hwdge queue in parallel with the
            # weight loads on the sync queue (halves the serial startup DMA) ----
            x_sb = xp.tile([P, CB, T], BF16, tag="x")
            nc.scalar.dma_start(x_sb[:, 0:3, :], xT[:, 0:3, :])
            nc.scalar.dma_start(x_sb[:, 3:CB, :], xT[:, 3:CB, :])
            wq_q = wqp.tile([P, CB, 2 * C], BF16, tag="wq")
            nc.sync.dma_start(wq_q[:, :, 0:C], wq16[:, :, 0:C])
            wv_q = wvp.tile([P, CB, C], BF16, tag="wv")
            nc.sync.dma_start(wv_q[:], wv16[:, :, :])
            nc.sync.dma_start(wq_q[:, :, C : 2 * C], wq16[:, :, C : 2 * C])
            wp_q = wpp.tile([P, CB, C], BF16, tag="wp")
            nc.sync.dma_start(wp_q[:], wp16[:, :, :])

            # ---- V-augmented tile ones column ----
            v_aug = vaugp.tile([P, 2 * KB, H, HD + 1], BF16, tag="vaug")
            nc.vector.tensor_copy(
                v_aug[:, :, :, HD : HD + 1],
                ones_col[:, None, :].to_broadcast([P, 2 * KB, H, 1]),
            )

            qksb = qksp.tile([P, MQK, T], BF16, tag="qksb")
            outT = outp.tile([P, CB, T], BF16, tag="outT")

            def emit_qkgen_quarter(mi, qc):
                """Q/K generation (bf16, unscaled ternary) for d-block mi,
                one 1024-token chunk, into qksb[:, mi, :]."""
                ps = psp.tile([P, 1024], F32, tag="ps")
                for half in range(2):
                    for ci in range(CB):
                        nc.tensor.matmul(
                            ps[:, half * 512 : (half + 1) * 512],
                            wq_q[:, ci, mi * P : (mi + 1) * P],
                            x_sb[:, ci,
                                 qc * 1024 + half * 512 : qc * 1024 + (half + 1) * 512],
                            start=(ci == 0),
                            stop=(ci == CB - 1),
                        )
                nc.scalar.activation(
                    qksb[:, mi, qc * 1024 : (qc + 1) * 1024], ps[:], AF.Copy
                )

            def emit_vgen_block(tb, nch):
                """V in natural layout (bf16) for one (token-block, half)."""
                ps = psp.tile([P, 1024], F32, tag="ps")
                for ci in range(CB):
                    nc.tensor.matmul(
                        ps[:, :384],
                        x_sb[:, ci, tb * P : (tb + 1) * P],
                        wv_q[:, ci, nch * 384 : (nch + 1) * 384],
                        start=(ci == 0),
                        stop=(ci == CB - 1),
                    )
                nc.vector.tensor_scalar_mul(
                    v_aug[:, tb, nch * 6 : (nch + 1) * 6, 0:HD],
                    ps[:, :384].rearrange("p (h d) -> p h d", d=HD),
                    sqb[:, 0:1],
                )

            def emit_proj(b, co):
                """proj output block co for batch b's tokens."""
                ps = psp.tile([P, 1024], F32, tag="ps")
                for half in range(2):
                    for ci in range(CB):
                        nc.tensor.matmul(
                            ps[:, half * 512 : (half + 1) * 512],
                            wp_q[:, ci, co * P : (co + 1) * P],
                            outT[:, ci, b * N + half * 512 : b * N + (half + 1) * 512],
                            start=(ci == 0),
                            stop=(ci == CB - 1),
                        )
                yst = ystp.tile([P, 1024], F32, tag="evac")
                nc.scalar.activation(
                    yst[:], ps[:], AF.Identity,
                    bias=b_sb[:, co : co + 1], scale=spb[:, 0:1],
                )
                nc.sync.dma_start(
                    yT[co, :, b * N : (b + 1) * N], yst[:]
                )

            def emit_attn(b, hp, filler, late=False, last=False):
                """attention for batch b, heads 2hp/2hp+1 (head-pair
                disjoint-row-groups, K=64); `filler`: zero-arg callables
                (PE work) spread across the kb loop."""
                avs = {}
                early_avsb = {}
                for hh in range(2):
                    avs[hh] = [
                        avp.tile([HD + 1, 512], F32, tag="av", name=f"av{hh}{qi}")
                        for qi in range(2)
                    ]
                nfill = len(filler)
                k = 0
                # front-load: half the filler in the first two kb steps to
                # cover the next-pair AV wait on av-slot recycling
                sched = [0.25, 0.5, 0.625, 0.75, 0.8125, 0.875, 0.9375, 1.0]
                if late:
                    sched = [0.0] * KB
                for kb in range(KB):
                    e2s = {}
                    for hh in range(2):
                        h = 2 * hp + hh
                        roff = hh * HD
                        st2 = psp.tile([P, 1024], F32, tag="ps", name=f"st{hh}")
                        for half in range(2):
                            nc.tensor.matmul(
                                st2[:, half * 512 : (half + 1) * 512],
                                qksb[roff : roff + HD, CB + hp,
                                     b * N + kb * P : b * N + (kb + 1) * P],
                                qksb[roff : roff + HD, hp,
                                     b * N + half * 512 : b * N + (half + 1) * 512],
                                start=True,
                                stop=True,
                            )
                        e2 = ep.tile([P, 1024], BF16, tag="e2", name=f"e{hh}")
                        nc.scalar.activation(
                            e2[:], st2[:], AF.Exp, bias=0.0, scale=sqb[:, 1:2]
                        )
                        e2s[hh] = e2
                        if _DEBUG and b == 0 and hp == 0 and hh == 0 and kb == 0:
                            nc.sync.dma_start(e_dbg[:, :], e2[:])
                    for hh in range(2):
                        h = 2 * hp + hh
                        for qi in range(2):
                            nc.tensor.matmul(
                                avs[hh][qi][:],
                                v_aug[:, b * KB + kb, h, :],
                                e2s[hh][:, qi * 512 : (qi + 1) * 512],
                                start=(kb == 0),
                                stop=(kb == KB - 1),
                            )
                        if kb == KB - 1 and not last and hh == 0:
                            # start evacuating head 0's accumulators while
                            # head 1's last AVs still stream
                            early_avsb[0] = avsp.tile(
                                [HD + 1, 1024], F32, tag="avsb", name="avsb0"
                            )
                            for qi in range(2):
                                nc.vector.tensor_copy(
                                    early_avsb[0][:, qi * 512 : (qi + 1) * 512],
                                    avs[0][qi][:],
                                )
                    while k < nfill and k + 1 <= sched[kb] * nfill:
                        filler[k]()
                        k += 1
                while k < nfill:
                    filler[k]()
                    k += 1
                # evacuate av accumulators to SBUF promptly so the PSUM
                # banks free for the next head-pair (the epilogue chain below
                # would otherwise hold them ~10us)
                if last:
                    # no later pair needs the av banks back: epilogue reads
                    # PSUM directly, skipping the evacuation copies
                    avsb = {
                        hh: {0: avs[hh][0], 1: avs[hh][1]} for hh in range(2)
                    }
                else:
                    avsb = {0: early_avsb[0]}
                    avsb[1] = avsp.tile(
                        [HD + 1, 1024], F32, tag="avsb", name="avsb1"
                    )
                    for qi in range(2):
                        nc.vector.tensor_copy(
                            avsb[1][:, qi * 512 : (qi + 1) * 512],
                            avs[1][qi][:],
                        )
                if _DEBUG and b == 0 and hp == 0:
                    nc.sync.dma_start(av_dbg[:, :], avsb[0][:, 0:512])
                # pack the 4 l rows at partition bases 0/32/64/96; one
                # reciprocal covers all four (DVE cost is free-size-driven)
                lsb4 = bcp.tile([P, 512], F32, tag="lraw")
                if memset_once[0] < 2:
                    nc.vector.memset(lsb4[:], 1.0)
                    memset_once[0] += 1
                for j, (hh, qi) in enumerate(
                    (hh, qi) for hh in range(2) for qi in range(2)
                ):
                    srcl = (
                        avsb[hh][qi][HD : HD + 1, :]
                        if last
                        else avsb[hh][HD : HD + 1, qi * 512 : (qi + 1) * 512]
                    )
                    nc.vector.tensor_copy(lsb4[32 * j : 32 * j + 1, :], srcl)
                linv4 = bcp.tile([P, 512], F32, tag="lsb")
                nc.vector.reciprocal(linv4[:], lsb4[:])
                ldram = dramls.tile([4, 512], F32, tag="ld")
                nc.sync.dma_start(
                    ldram[:], linv4[:, :].rearrange("(f p) c -> f p c", p=32)[:, 0, :]
                )
                if _DEBUG and b == 0 and hp == 0:
                    nc.sync.dma_start(linv_dbg[:, :], linv4[0:1, :])
                for j, (hh, qi) in enumerate(
                    (hh, qi) for hh in range(2) for qi in range(2)
                ):
                    h = 2 * hp + hh
                    bc = bcp.tile([HD, 512], F32, tag="bc")
                    eng = nc.sync if j % 2 == 0 else nc.gpsimd
                    eng.dma_start(
                        bc[:], ldram[j : j + 1, :].to_broadcast([HD, 512])
                    )
                    if _DEBUG and b == 0 and hp == 0 and j == 0:
                        nc.sync.dma_start(bc_dbg[:, :], bc[:])
                    srcm = (
                        avsb[hh][qi][0:HD, :]
                        if last
                        else avsb[hh][0:HD, qi * 512 : (qi + 1) * 512]
                    )
                    nc.vector.tensor_mul(
                        out=outT[
                            (h % 2) * HD : (h % 2) * HD + HD,
                            h // 2,
                            b * N + qi * 512 : b * N + (qi + 1) * 512,
                        ],
                        in0=srcm,
                        in1=bc[:],
                    )

            # ---------------- emission schedule ----------------
            for qc in range(2):
                for mi in (0, CB):
                    emit_qkgen_quarter(mi, qc)
            for tb in range(KB):
                for nch in range(2):
                    emit_vgen_block(tb, nch)

            memset_once = [0]

            # b=0 head pairs; filler: next pair's Q/K-gen (or V-gen b1)
            for hp in range(HP):
                filler = []
                if hp + 1 < HP:
                    for mi in (hp + 1, CB + hp + 1):
                        for qc in range(2):
                            filler.append(
                                lambda mi=mi, qc=qc: emit_qkgen_quarter(mi, qc)
                            )
                else:
                    for tb in range(KB, 2 * KB):
                        for nch in range(2):
                            filler.append(
                                lambda tb=tb, nch=nch: emit_vgen_block(tb, nch)
                            )
                emit_attn(0, hp, filler)

            # b=1 head pairs; filler: proj b=0. The last co block is held
            # back as late filler so it drains during the final pair's
            # epilogue instead of leaving the PE idle
            for hp in range(HP):
                fl = [lambda co=hp: emit_proj(0, co)]
                emit_attn(1, hp, fl, late=(hp == HP - 1), last=(hp == HP - 1))

            for co in range(CB):
                emit_proj(1, co)

            if _DEBUG:
                nc.sync.dma_start(qk_dbg[:, :, :], qksb[:])
                nc.sync.dma_start(va_dbg[:, :, :, :], v_aug[:])
                nc.sync.dma_start(out_dbg[:, :, :], outT[:])

    nc.finalize()
    return nc


def _get_nc(split=True):
    global _CACHED_NC
    if _CACHED_NC is None:
        _CACHED_NC = _build_nc(split=split)
    return _CACHED_NC


def _ternary(w):
    """Host-side ternary quantization matching the reference's boundary
    decisions: s/thr in float64, comparisons on the float32 weights."""
    w = np.asarray(w, dtype=np.float32)
    s64 = np.float64(np.mean(np.abs(w), dtype=np.float64))
    s = np.float32(s64)
    thr = np.float32(0.5) * (s + np.float32(EPS))
    t = (w > thr).astype(np.float32) - (w < -thr).astype(np.float32)
    return t, s


def run(x, w_qkv, w_proj, b_proj, trace=False):
    x = np.ascontiguousarray(x, dtype=np.float32)
    tq, s_q = _ternary(w_qkv)    # [3C, C]
    tp, s_p = _ternary(w_proj)   # [C, C]
    bp = np.ascontiguousarray(b_proj, dtype=np.float32)
    es = np.float32(SCALE) * s_q * s_q
    sq = np.array([[s_q, es]], dtype=np.float32)
    sp = np.array([[s_p]], dtype=np.float32)
    cz_host = np.zeros((2, N), dtype=ml_dtypes.bfloat16)
    cz_host[1, :] = 1.0

    tqT = np.ascontiguousarray(tq.T)  # [C, 3C]
    wq16 = np.ascontiguousarray(
        tqT[:, : 2 * C].reshape(CB, P, 2 * C).transpose(1, 0, 2)
    ).astype(ml_dtypes.bfloat16)
    wv16 = np.ascontiguousarray(
        tqT[:, 2 * C :].reshape(CB, P, C).transpose(1, 0, 2)
    ).astype(ml_dtypes.bfloat16)
    wp16 = np.ascontiguousarray(
        np.ascontiguousarray(tp.T).reshape(CB, P, C).transpose(1, 0, 2)
    ).astype(ml_dtypes.bfloat16)

    in_maps = []
    for c in range(NCORES):
        xs = x[c * BPC : (c + 1) * BPC].reshape(T, C)
        # pre-arranged [P, CB, T]: xT[p, cb, t] = x[t, cb*128 + p]
        xsT = np.ascontiguousarray(
            xs.T.reshape(CB, P, T).transpose(1, 0, 2)
        )
        in_maps.append(
            {
                "xT": xsT.astype(ml_dtypes.bfloat16),
                "wq16": wq16,
                "wv16": wv16,
                "wp16": wp16,
                "bp": bp,
                "sq": sq,
                "sp": sp,
                "cz": cz_host,
            }
        )

    nc = _get_nc()
    res = run_bass_kernel_spmd(
        nc, in_maps, core_ids=list(range(NCORES)), trace=trace
    )

    y = np.empty((B, N, C), dtype=np.float32)
    for c in range(NCORES):
        yT_c = res.results[c]["yT"].reshape(C, T)  # [CB, P, T] -> [C, T]
        y[c * BPC : (c + 1) * BPC] = yT_c.T.reshape(BPC, N, C)
    return y, res


def run_debug(x, w_qkv, w_proj, b_proj):
    global _DEBUG, _CACHED_NC
    _DEBUG = True
    _CACHED_NC = None
    try:
        return run(x, w_qkv, w_proj, b_proj, trace=False)
    finally:
        _DEBUG = False
        _CACHED_NC = None


def kernel(x, w_qkv, w_proj, b_proj):
    y, _ = run(x, w_qkv, w_proj, b_proj, trace=False)
    return y

